# revision 1
# baseline (speedup 1.0000x reference)
"""Trainium2 Bass kernel for sliding-window GQA attention (qk-norm + RoPE).

Problem: B=2, S=2048, D=1024, 16 heads / 4 kv heads, head_dim 64,
causal sliding window 512, fp32 I/O.

Sharding: 8 cores = batch(2) x sequence(4). Each core computes 512 query
tokens against a 1024-token context window (512-token halo; chunk 0 is
zero-padded on the left). Fully data-parallel SPMD - no collectives.

On-chip dataflow (per core) is built around a transposed score layout so
that no softmax transposes are needed:
  xT (host-pre-transposed) -> q/k/v projections in [token, dim] layout
  -> rmsnorm + RoPE (tables host-precomputed, qn/kn weights folded in)
  -> PE-transpose q/k to [dim, token] -> scores sT[k, q] per 128-token
  block -> exp on ACT -> two static triangle masks for the sliding
  window -> attn @ v with an extra ones-column producing the softmax
  denominator in the same matmul -> per-head normalize -> out-proj.

Head-slot permutation: q heads are permuted on the host so every head's
64 q-rows sit at the same SBUF partition offset (0 or 64) as its kv
group's k-rows - matmul requires lhsT/rhs base partitions to match.
wo rows are permuted to match. The within-head dims of q/k are permuted
evens-first so RoPE becomes two contiguous 32-wide halves (scores are
invariant to a shared q/k dim permutation).
"""

import sys

sys.path.insert(0, "/opt/trn_rl_repo")

from contextlib import ExitStack

import numpy as np
import ml_dtypes

import bass_rust
import concourse.bass as bass
import concourse.tile as tile
from concourse import mybir

# ---------------- problem constants ----------------
B, S, D = 2, 2048, 1024
H, KV, HD = 16, 4, 64
WINDOW = 512
EPS = 1e-5
NCORES = 8
TQ = 512          # query tokens per core
TC = 1024         # context tokens per core (incl. 512 halo)
NQT = TQ // 128   # 4 query tiles
NCT = TC // 128   # 8 context tiles
P = 128

F32 = mybir.dt.float32
BF16 = mybir.dt.bfloat16
F32R = mybir.dt.float32r
ALU = mybir.AluOpType
ACTF = mybir.ActivationFunctionType

# q-head -> slot permutation with parity matching:
# slot p must satisfy p%2 == (head//4)%2 so that the q rows (at partition
# offset (p%2)*64) align with the kv group's k rows.
HEAD_OF_SLOT = [0, 4, 1, 5, 2, 6, 3, 7, 8, 12, 9, 13, 10, 14, 11, 15]


def split_multiwaits(nc):
    """This environment's walrus build rejects any instruction with more
    than one sync-wait condition. Split extras into preceding single-wait
    NoOps on the same engine (identical blocking semantics)."""
    n_split = 0
    for f in nc.m.functions:
        for blk in f.blocks:
            out = []
            changed = False
            for inst in blk.instructions:
                try:
                    si = inst.sync_info
                    waits = list(si.on_wait)
                except Exception:
                    out.append(inst)
                    continue
                if len(waits) > 1:
                    changed = True
                    for j, w in enumerate(waits[:-1]):
                        nop = mybir.InstNoOp(
                            name=f"{inst.name}-wsplit{j}", ins=[], outs=[])
                        nop.engine = inst.engine
                        nop.sync_info = bass_rust.SyncInfo(
                            on_wait=[w], on_update=[])
                        nc.register_instruction(nop, overwrite=True)
                        out.append(nop)
                        n_split += 1
                    inst.sync_info = bass_rust.SyncInfo(
                        on_wait=[waits[-1]], on_update=list(si.on_update))
                out.append(inst)
            if changed:
                blk.instructions = out
    return n_split


# ---------------- program builder ----------------

def emit(nc, tc, ctx):
    cp = ctx.enter_context(tc.tile_pool(name="const", bufs=1))
    pp = ctx.enter_context(tc.tile_pool(name="pp", bufs=2, space="PSUM"))
    ntp = ctx.enter_context(tc.tile_pool(name="ntp", bufs=2, space="PSUM"))
    stp = ctx.enter_context(tc.tile_pool(name="stp", bufs=2, space="PSUM"))
    scr = ctx.enter_context(tc.tile_pool(name="scr", bufs=4))
    epool = ctx.enter_context(tc.tile_pool(name="epool", bufs=4))
    rpool = ctx.enter_context(tc.tile_pool(name="rpool", bufs=3))

    # DRAM params
    xt_d = nc.declare_dram_parameter("xt", [D, TC], BF16, isOutput=False)
    wq_d = nc.declare_dram_parameter("wq", [D, H * HD], BF16, isOutput=False)
    wk_d = nc.declare_dram_parameter("wk", [D, KV * HD], BF16, isOutput=False)
    wv_d = nc.declare_dram_parameter("wv", [D, KV * HD], BF16, isOutput=False)
    wo_d = nc.declare_dram_parameter("wo", [H * HD, D], BF16, isOutput=False)
    cosq_d = nc.declare_dram_parameter("cosq", [P, NQT, HD], BF16, isOutput=False)
    sinq_d = nc.declare_dram_parameter("sinq", [P, NQT, HD], BF16, isOutput=False)
    cosk_d = nc.declare_dram_parameter("cosk", [P, NCT, HD], BF16, isOutput=False)
    sink_d = nc.declare_dram_parameter("sink", [P, NCT, HD], BF16, isOutput=False)
    vmask_d = nc.declare_dram_parameter("vmask", [P, NCT], F32, isOutput=False)
    y_d = nc.declare_dram_parameter("y", [TQ, D], F32, isOutput=True)

    # persistent SBUF
    xt = cp.tile([P, 8, TC], BF16, tag="xt")
    wq = cp.tile([P, 8, 1024], BF16, tag="wq")
    wk = cp.tile([P, 8, 256], BF16, tag="wk")
    wv = cp.tile([P, 8, 256], BF16, tag="wv")
    wo = cp.tile([P, 8, 1024], BF16, tag="wo")
    cosq = cp.tile([P, NQT, HD], BF16, tag="cosq")
    sinq = cp.tile([P, NQT, HD], BF16, tag="sinq")
    cosk = cp.tile([P, NCT, HD], BF16, tag="cosk")
    sink = cp.tile([P, NCT, HD], BF16, tag="sink")
    vmask = cp.tile([P, NCT], F32, tag="vmask")
    qT = cp.tile([P, 8, TQ], BF16, tag="qT")       # [j, jt, a]
    kT = cp.tile([P, 2, TC], BF16, tag="kT")       # [j, jt2, p]
    vA = cp.tile([P, NCT, KV, 65], BF16, tag="vA")  # v | valid-col @64
    vB = cp.tile([P, NCT, KV, 128], BF16, tag="vB")  # zeros | valid@32 | v@64:
    q_raw = cp.tile([P, NQT, 1024], BF16, tag="qraw")
    qrot = cp.tile([P, NQT, 1024], BF16, tag="qrot")
    k_raw = cp.tile([P, NCT, 256], BF16, tag="kraw")
    krot = cp.tile([P, NCT, 256], BF16, tag="krot")
    oT = cp.tile([P, 8, TQ], BF16, tag="oT")
    y_sb = cp.tile([P, NQT, 1024], F32, tag="ysb")
    ident = cp.tile([P, P], BF16, tag="ident")
    mT0 = cp.tile([P, P], BF16, tag="mT0")
    mT4 = cp.tile([P, P], BF16, tag="mT4")
    onesb = cp.tile([P, P], F32, tag="onesb")
    den_all = cp.tile([P, 2, 8, P], F32, tag="den")
    rec_all = cp.tile([P, 2, 8, P], F32R, tag="recall")
    ssq_q = cp.tile([P, NQT, H], F32, tag="ssqq")
    ssq_k = cp.tile([P, NCT, KV], F32, tag="ssqk")

    # ---- input DMAs ----
    nc.sync.dma_start(cosk[:], cosk_d[:])
    nc.sync.dma_start(sink[:], sink_d[:])
    nc.sync.dma_start(cosq[:], cosq_d[:])
    nc.sync.dma_start(sinq[:], sinq_d[:])
    nc.sync.dma_start(vmask[:], vmask_d[:])
    nc.sync.dma_start(xt[:], xt_d.rearrange("(a p) t -> p a t", p=P))
    nc.sync.dma_start(wk[:], wk_d.rearrange("(a p) n -> p a n", p=P))
    nc.sync.dma_start(wv[:], wv_d.rearrange("(a p) n -> p a n", p=P))
    nc.sync.dma_start(wq[:], wq_d.rearrange("(a p) n -> p a n", p=P))

    # ---- on-chip constants ----
    # identity for PE transposes
    nc.gpsimd.memset(ident[:], 0.0)
    nc.gpsimd.affine_select(
        out=ident[:], in_=ident[:], compare_op=ALU.not_equal, fill=1.0,
        base=0, pattern=[[-1, P]], channel_multiplier=1)
    # additive sliding-window masks, applied on the PE as an extra accumulate
    # matmul (out += mT.T @ I adds the constant tile into the score PSUM).
    # slot 0 keeps a < kp: add -30000 where a >= kp -> mT0[x,y] = -3e4 iff x>=y
    nc.gpsimd.memset(mT0[:], 0.0)
    nc.gpsimd.affine_select(
        out=mT0[:], in_=mT0[:], compare_op=ALU.is_gt, fill=-30000.0,
        base=0, pattern=[[1, P]], channel_multiplier=-1)
    # slot 4 keeps a >= kp: add -30000 where a < kp -> mT4[x,y] = -3e4 iff x<y
    nc.gpsimd.memset(mT4[:], 0.0)
    nc.gpsimd.affine_select(
        out=mT4[:], in_=mT4[:], compare_op=ALU.is_ge, fill=-30000.0,
        base=0, pattern=[[-1, P]], channel_multiplier=1)
    # v augmentation fixed columns: the "ones" column is the per-context-tile
    # validity (0 for left-pad tiles on chunk 0) so padded keys contribute
    # nothing to the softmax denominator.
    nc.gpsimd.memset(vB[:], 0.0)
    nc.gpsimd.memset(onesb[:], 1.0)
    for g in range(KV):
        nc.vector.tensor_copy(vA[:, :, g, 64:65], vmask[:].unsqueeze(2))
        nc.vector.tensor_copy(vB[:, :, g, 32:33], vmask[:].unsqueeze(2))

    inv64 = 1.0 / 64.0

    def rmsnorm_rope(raw, rot, nt, nh, ssq, cosT, sinT, it):
        """raw/rot: [P, nt, nh*64] bf16 slabs; process tile `it`."""
        hv = raw[:, it].rearrange("p (h d) -> p h d", h=nh)
        rv = rot[:, it].rearrange("p (h d) -> p h d", h=nh)
        s64 = scr.tile([P, HD], BF16, tag="s64")
        for h in range(nh):
            nc.vector.scalar_tensor_tensor(
                out=s64[:], in0=hv[:, h], scalar=1.0, in1=hv[:, h],
                op0=ALU.mult, op1=ALU.mult,
                accum_out=ssq[:, it, h:h + 1])
        ms = scr.tile([P, nh], F32, tag="ms")
        nc.vector.tensor_scalar(
            out=ms[:], in0=ssq[:, it], scalar1=inv64, scalar2=EPS,
            op0=ALU.mult, op1=ALU.add)
        sq = scr.tile([P, nh], F32, tag="sq")
        nc.scalar.sqrt(sq[:], ms[:])
        rsf = scr.tile([P, nh], F32, tag="rsf")
        nc.vector.reciprocal(rsf[:], sq[:])
        rsb = scr.tile([P, nh], BF16, tag="rsb")
        nc.vector.tensor_copy(rsb[:], rsf[:])
        # normalize in place
        nc.vector.tensor_tensor(
            out=hv[:], in0=hv[:],
            in1=rsb[:].unsqueeze(2).broadcast_to([P, nh, HD]),
            op=ALU.mult)
        # rope: halves are contiguous thanks to the host evens-first permute
        yA = hv[:, :, 0:32]
        yB = hv[:, :, 32:64]
        cA = cosT[:, it:it + 1, 0:32].broadcast_to([P, nh, 32])
        cB = cosT[:, it:it + 1, 32:64].broadcast_to([P, nh, 32])
        sA = sinT[:, it:it + 1, 0:32].broadcast_to([P, nh, 32])
        sB = sinT[:, it:it + 1, 32:64].broadcast_to([P, nh, 32])
        r1 = scr.tile([P, 512], BF16, tag="r1")
        r2 = scr.tile([P, 512], BF16, tag="r2")
        w = nh * 32
        r1v = r1[:, 0:w].rearrange("p (h d) -> p h d", h=nh)
        r2v = r2[:, 0:w].rearrange("p (h d) -> p h d", h=nh)
        nc.vector.tensor_mul(r1v[:], yA, cA)
        nc.vector.tensor_mul(r2v[:], yB, sA)
        nc.gpsimd.tensor_tensor(out=rv[:, :, 0:32], in0=r1v[:], in1=r2v[:],
                                op=ALU.subtract)
        nc.gpsimd.tensor_mul(r1v[:], yB, cB)
        nc.gpsimd.tensor_mul(r2v[:], yA, sB)
        nc.gpsimd.tensor_tensor(out=rv[:, :, 32:64], in0=r1v[:], in1=r2v[:],
                                op=ALU.add)

    # ---- k/v projections + norm/rope over the 8 context tiles ----
    for ct in range(NCT):
        kps = pp.tile([P, 512], F32, tag="pp")
        vps = pp.tile([P, 512], F32, tag="pp")
        for dt in range(8):
            lhs = xt[:, dt, ct * P:(ct + 1) * P]
            nc.tensor.matmul(kps[:, 0:256], lhs, wk[:, dt],
                             start=(dt == 0), stop=(dt == 7))
            nc.tensor.matmul(vps[:, 0:256], lhs, wv[:, dt],
                             start=(dt == 0), stop=(dt == 7))
        nc.scalar.copy(k_raw[:, ct], kps[:, 0:256])
        # v -> vA (cols 0:64 per group) and vB (cols 64:128)
        vAv = vA[:, ct].rearrange("p g d -> p (g d)")
        nc.scalar.copy(
            vA[:, ct, :, 0:64], vps[:, 0:256].rearrange("p (g d) -> p g d", g=KV))
        nc.scalar.copy(
            vB[:, ct, :, 64:128], vps[:, 0:256].rearrange("p (g d) -> p g d", g=KV))
        rmsnorm_rope(k_raw, krot, NCT, KV, ssq_k, cosk, sink, ct)
        # transpose krot tile -> kT
        for j2 in range(2):
            tp = stp.tile([P, P], BF16, tag="stp")
            nc.tensor.transpose(tp[:], krot[:, ct, j2 * P:(j2 + 1) * P], ident[:])
            nc.vector.tensor_copy(kT[:, j2, ct * P:(ct + 1) * P], tp[:])

    # ---- q projection + norm/rope over the 4 query tiles ----
    for at in range(NQT):
        qps0 = pp.tile([P, 512], F32, tag="pp")
        qps1 = pp.tile([P, 512], F32, tag="pp")
        for dt in range(8):
            lhs = xt[:, dt, TQ + at * P:TQ + (at + 1) * P]
            nc.tensor.matmul(qps0[:], lhs, wq[:, dt, 0:512],
                             start=(dt == 0), stop=(dt == 7))
            nc.tensor.matmul(qps1[:], lhs, wq[:, dt, 512:1024],
                             start=(dt == 0), stop=(dt == 7))
        nc.scalar.copy(q_raw[:, at, 0:512], qps0[:])
        nc.scalar.copy(q_raw[:, at, 512:1024], qps1[:])
        rmsnorm_rope(q_raw, qrot, NQT, H, ssq_q, cosq, sinq, at)
        for jt in range(8):
            tp = stp.tile([P, P], BF16, tag="stp")
            nc.tensor.transpose(tp[:], qrot[:, at, jt * P:(jt + 1) * P], ident[:])
            nc.vector.tensor_copy(qT[:, jt, at * P:(at + 1) * P], tp[:])

    # ---- attention ----
    for qb in range(NQT):
        for p_slot in range(H):
            g = HEAD_OF_SLOT[p_slot] // 4
            par = p_slot % 2            # == g % 2 by construction
            off = par * 64
            sT = stp.tile([P, 5, P], F32, tag="stp")
            for s_ in range(5):
                kt = qb + s_
                masked = s_ in (0, 4)
                nc.tensor.matmul(
                    sT[:, s_, :],
                    kT[off:off + 64, g // 2, kt * P:(kt + 1) * P],
                    qT[off:off + 64, p_slot // 2, qb * P:(qb + 1) * P],
                    start=True, stop=not masked)
                if masked:
                    nc.tensor.matmul(
                        sT[:, s_, :], mT0[:] if s_ == 0 else mT4[:], ident[:],
                        start=False, stop=True)
            e = epool.tile([P, 5, P], BF16, tag="e")
            nc.scalar.activation(e[:], sT[:], ACTF.Exp, bias=0.0,
                                 scale=float(HD) ** -0.5)
            nT = ntp.tile([P, P], F32, tag="ntp")
            for s_ in range(5):
                kt = qb + s_
                if par == 0:
                    lhsT = vA[:, kt, g]          # [128, 65] -> out rows 0..64
                else:
                    lhsT = vB[:, kt, g]          # [128, 128] -> v at 64.., den@32
                nc.tensor.matmul(nT[0:lhsT.shape[-1], :], lhsT, e[:, s_, :],
                                 start=(s_ == 0), stop=(s_ == 4))
            dp = 64 if par == 0 else 32
            nc.vector.tensor_copy(
                den_all[dp:dp + 1, par, p_slot // 2, :], nT[dp:dp + 1, :])
            nc.vector.tensor_copy(
                oT[off:off + 64, p_slot // 2, qb * P:(qb + 1) * P],
                nT[off:off + 64, :])
        # batched reciprocal of the 16 denominators (DMA partition-gather),
        # then f32r K=1 outer products broadcast 1/den to 64 rows x 4 heads
        ge = rpool.tile([16, P], F32, tag="ge")
        nc.sync.dma_start(ge[0:8, :], den_all[64:65, 0, :, :])
        nc.sync.dma_start(ge[8:16, :], den_all[32:33, 1, :, :])
        gr = rpool.tile([16, P], F32, tag="gr")
        nc.vector.reciprocal(gr[:], ge[:])
        nc.sync.dma_start(rec_all[64:65, 0, :, :], gr[0:8, :].bitcast(F32R))
        nc.sync.dma_start(rec_all[64:65, 1, :, :], gr[8:16, :].bitcast(F32R))
        for par in range(2):
            off = 0 if par == 0 else 64
            for half in range(2):
                rb = pp.tile([P, 512], F32, tag="pp")
                nc.tensor.matmul(
                    rb[:, :], onesb[64:65, :].bitcast(F32R),
                    rec_all[64:65, par, half * 4:(half + 1) * 4, :],
                    start=True, stop=True)
                rbsb = rpool.tile([P, 512], F32, tag="rbsb")
                nc.scalar.copy(rbsb[off:off + 64, :], rb[off:off + 64, :])
                for i in range(4):
                    p_slot = (half * 4 + i) * 2 + par
                    nc.vector.tensor_mul(
                        oT[off:off + 64, p_slot // 2, qb * P:(qb + 1) * P],
                        oT[off:off + 64, p_slot // 2, qb * P:(qb + 1) * P],
                        rbsb[off:off + 64, i * P:(i + 1) * P])

    # ---- out-projection ----
    nc.sync.dma_start(wo[:], wo_d.rearrange("(a p) n -> p a n", p=P))
    for at in range(NQT):
        for dh in range(2):
            yps = pp.tile([P, 512], F32, tag="pp")
            for jt in range(8):
                nc.tensor.matmul(
                    yps[:],
                    oT[:, jt, at * P:(at + 1) * P],
                    wo[:, jt, dh * 512:(dh + 1) * 512],
                    start=(jt == 0), stop=(jt == 7))
            if dh == 0:
                nc.scalar.copy(y_sb[:, at, 0:512], yps[:])
            else:
                nc.vector.tensor_copy(y_sb[:, at, 512:1024], yps[:])

    nc.sync.dma_start(y_d.rearrange("(a p) n -> p a n", p=P), y_sb[:])


def build_program():
    nc = bass.Bass()
    with tile.TileContext(nc) as tc:
        with ExitStack() as ctx:
            emit(nc, tc, ctx)
    split_multiwaits(nc)
    return nc


_NC = None


def _get_program():
    global _NC
    if _NC is None:
        _NC = build_program()
    return _NC


# ---------------- host-side prep ----------------

def prep_core_inputs(x, wq, wk, wv, wo, qn_w, kn_w):
    bf = ml_dtypes.bfloat16
    perm = np.concatenate([np.arange(0, 64, 2), np.arange(1, 64, 2)])

    wq_p = np.ascontiguousarray(
        wq.reshape(D, H, HD)[:, HEAD_OF_SLOT][:, :, perm].reshape(D, H * HD)
    ).astype(bf)
    wk_p = np.ascontiguousarray(
        wk.reshape(D, KV, HD)[:, :, perm].reshape(D, KV * HD)).astype(bf)
    wv_p = np.ascontiguousarray(wv).astype(bf)
    wo_p = np.ascontiguousarray(
        wo.reshape(H, HD, D)[HEAD_OF_SLOT].reshape(H * HD, D)).astype(bf)

    inv_freq = 1.0 / (10000.0 ** (np.arange(0, HD, 2, dtype=np.float64) / HD))
    freq64 = np.concatenate([inv_freq, inv_freq])  # emb[t, d] = t * freq64[d]

    def rope_tables(tvec, w):
        ang = tvec[:, None].astype(np.float64) * freq64[None, :]
        c = np.cos(ang).astype(np.float32)
        s_ = np.sin(ang).astype(np.float32)
        we, wo_ = w[0::2], w[1::2]
        cosT = np.concatenate([we[None] * c[:, 0::2], wo_[None] * c[:, 1::2]], axis=1)
        sinT = np.concatenate([wo_[None] * s_[:, 0::2], we[None] * s_[:, 1::2]], axis=1)
        return cosT, sinT

    in_maps = []
    for core in range(NCORES):
        b, ci = divmod(core, NCORES // B)
        q_lo = ci * TQ
        c_lo = q_lo - WINDOW
        ctx_blk = np.zeros((TC, D), np.float32)
        lo = max(c_lo, 0)
        ctx_blk[lo - c_lo:, :] = x[b, lo:q_lo + TQ, :]
        xt_c = np.ascontiguousarray(ctx_blk.T).astype(bf)

        tq = np.arange(q_lo, q_lo + TQ)
        cq, sq = rope_tables(tq, qn_w)
        cosq_c = np.ascontiguousarray(
            cq.reshape(NQT, P, HD).transpose(1, 0, 2)).astype(bf)
        sinq_c = np.ascontiguousarray(
            sq.reshape(NQT, P, HD).transpose(1, 0, 2)).astype(bf)
        tk = np.arange(c_lo, c_lo + TC)
        ck, sk = rope_tables(tk, kn_w)
        cosk_c = np.ascontiguousarray(
            ck.reshape(NCT, P, HD).transpose(1, 0, 2)).astype(bf)
        sink_c = np.ascontiguousarray(
            sk.reshape(NCT, P, HD).transpose(1, 0, 2)).astype(bf)

        vm = np.ones((P, NCT), np.float32)
        n_pad_tiles = (lo - c_lo) // P
        vm[:, :n_pad_tiles] = 0.0

        in_maps.append({
            "xt": xt_c, "wq": wq_p, "wk": wk_p, "wv": wv_p, "wo": wo_p,
            "cosq": cosq_c, "sinq": sinq_c, "cosk": cosk_c, "sink": sink_c,
            "vmask": vm,
        })
    return in_maps


def kernel(x, wq, wk, wv, wo, qn_w, kn_w):
    from concourse.bass_utils import run_bass_kernel_spmd
    in_maps = prep_core_inputs(x, wq, wk, wv, wo, qn_w, kn_w)
    nc = _get_program()
    res = run_bass_kernel_spmd(nc, in_maps, list(range(NCORES)))
    out = np.empty((B, S, D), np.float32)
    for core in range(NCORES):
        b, ci = divmod(core, NCORES // B)
        out[b, ci * TQ:(ci + 1) * TQ, :] = res.results[core]["y"]
    return out



# revision 32
# speedup vs baseline: 344.7726x; 344.7726x over previous
"""Trainium2 Bass kernel for sliding-window GQA attention (qk-norm + RoPE).

Problem: B=2, S=2048, D=1024, 16 heads / 4 kv heads, head_dim 64,
causal sliding window 512, fp32 I/O.

Sharding: 8 cores = batch(2) x sequence(4). Each core computes 512 query
tokens against a 1024-token context window (512-token halo; chunk 0 is
zero-padded on the left). Fully data-parallel SPMD - no collectives.

On-chip dataflow (per core) is built around a transposed score layout so
that no softmax transposes are needed:
  xT (host-pre-transposed) -> q/k/v projections in [token, dim] layout
  -> rmsnorm + RoPE (tables host-precomputed, qn/kn weights folded in)
  -> PE-transpose q/k to [dim, token] -> scores sT[k, q] per 128-token
  block -> exp on ACT -> two static triangle masks for the sliding
  window -> attn @ v with an extra ones-column producing the softmax
  denominator in the same matmul -> per-head normalize -> out-proj.

Head-slot permutation: q heads are permuted on the host so every head's
64 q-rows sit at the same SBUF partition offset (0 or 64) as its kv
group's k-rows - matmul requires lhsT/rhs base partitions to match.
wo rows are permuted to match. The within-head dims of q/k are permuted
evens-first so RoPE becomes two contiguous 32-wide halves (scores are
invariant to a shared q/k dim permutation).
"""

import sys

sys.path.insert(0, "/opt/trn_rl_repo")

from contextlib import ExitStack

import numpy as np
import ml_dtypes

import bass_rust
import concourse.bass as bass
import concourse.tile as tile
from concourse import mybir

# ---------------- problem constants ----------------
B, S, D = 2, 2048, 1024
H, KV, HD = 16, 4, 64
WINDOW = 512
EPS = 1e-5
NCORES = 8
TQ = 512          # query tokens per core
TC = 1024         # context tokens per core (incl. 512 halo)
NQT = TQ // 128   # 4 query tiles
NCT = TC // 128   # 8 context tiles
P = 128

F32 = mybir.dt.float32
BF16 = mybir.dt.bfloat16
F32R = mybir.dt.float32r
ALU = mybir.AluOpType
ACTF = mybir.ActivationFunctionType

# q-head -> slot permutation with parity matching:
# slot p must satisfy p%2 == (head//4)%2 so that the q rows (at partition
# offset (p%2)*64) align with the kv group's k rows.
HEAD_OF_SLOT = [0, 4, 1, 5, 2, 6, 3, 7, 8, 12, 9, 13, 10, 14, 11, 15]


def split_multiwaits(nc):
    """This environment's walrus build rejects any instruction with more
    than one sync-wait condition. Split extras into preceding single-wait
    NoOps on the same engine (identical blocking semantics)."""
    n_split = 0
    for f in nc.m.functions:
        for blk in f.blocks:
            out = []
            changed = False
            for inst in blk.instructions:
                try:
                    si = inst.sync_info
                    waits = list(si.on_wait)
                except Exception:
                    out.append(inst)
                    continue
                if len(waits) > 1:
                    changed = True
                    for j, w in enumerate(waits[:-1]):
                        nop = mybir.InstNoOp(
                            name=f"{inst.name}-wsplit{j}", ins=[], outs=[])
                        nop.engine = inst.engine
                        nop.sync_info = bass_rust.SyncInfo(
                            on_wait=[w], on_update=[])
                        nc.register_instruction(nop, overwrite=True)
                        out.append(nop)
                        n_split += 1
                    inst.sync_info = bass_rust.SyncInfo(
                        on_wait=[waits[-1]], on_update=list(si.on_update))
                out.append(inst)
            if changed:
                blk.instructions = out
    return n_split


# ---------------- program builder ----------------

def emit(nc, tc, ctx, stage=3):
    """stage (timing-only knob): 0 = input DMAs only, 1 = + projections/
    norm/rope/transposes, 2 = + attention, 3 = full (+ out-projection).
    stage<3 leaves y_sb unwritten - outputs are garbage, timing variants
    only."""
    cp = ctx.enter_context(tc.tile_pool(name="const", bufs=1))
    pp = ctx.enter_context(tc.tile_pool(name="pp", bufs=2, space="PSUM"))
    ntp = ctx.enter_context(tc.tile_pool(name="ntp", bufs=2, space="PSUM"))
    stp = ctx.enter_context(tc.tile_pool(name="stp", bufs=4, space="PSUM"))
    scr = ctx.enter_context(tc.tile_pool(name="scr", bufs=4))
    epool = ctx.enter_context(tc.tile_pool(name="epool", bufs=4))
    rpool = ctx.enter_context(tc.tile_pool(name="rpool", bufs=3))

    # DRAM params
    xt_d = nc.declare_dram_parameter("xt", [D, TC], BF16, isOutput=False)
    wq_d = nc.declare_dram_parameter("wq", [D, H * HD], BF16, isOutput=False)
    wk_d = nc.declare_dram_parameter("wk", [D, KV * HD], BF16, isOutput=False)
    wv_d = nc.declare_dram_parameter("wv", [D, KV * HD], BF16, isOutput=False)
    wo_d = nc.declare_dram_parameter("wo", [H * HD, D], BF16, isOutput=False)
    cosq_d = nc.declare_dram_parameter("cosq", [P, NQT, HD], BF16, isOutput=False)
    sinq_d = nc.declare_dram_parameter("sinq", [P, NQT, HD], BF16, isOutput=False)
    cosk_d = nc.declare_dram_parameter("cosk", [P, NCT, HD], BF16, isOutput=False)
    sink_d = nc.declare_dram_parameter("sink", [P, NCT, HD], BF16, isOutput=False)
    vmask_d = nc.declare_dram_parameter("vmask", [P, NCT], F32, isOutput=False)
    y_d = nc.declare_dram_parameter("y", [TQ, D], F32, isOutput=True)

    # persistent SBUF
    xt = cp.tile([P, 8, TC], BF16, tag="xt")
    wq = cp.tile([P, 8, 1024], BF16, tag="wq")
    wk = cp.tile([P, 8, 256], BF16, tag="wk")
    wv = cp.tile([P, 8, 256], BF16, tag="wv")
    wo = cp.tile([P, 8, 1024], BF16, tag="wo")
    cosq = cp.tile([P, NQT, HD], BF16, tag="cosq")
    sinq = cp.tile([P, NQT, HD], BF16, tag="sinq")
    cosk = cp.tile([P, NCT, HD], BF16, tag="cosk")
    sink = cp.tile([P, NCT, HD], BF16, tag="sink")
    vmask = cp.tile([P, NCT], F32, tag="vmask")
    qT = cp.tile([P, 8, TQ], BF16, tag="qT")       # [j, jt, a]
    kT = cp.tile([P, 2, TC], BF16, tag="kT")       # [j, jt2, p]
    vA = cp.tile([P, NCT, KV, 65], BF16, tag="vA")  # v | valid-col @64
    vB = cp.tile([P, NCT, KV, 128], BF16, tag="vB")  # zeros | valid@32 | v@64:
    q_raw = cp.tile([P, NQT, 1024], BF16, tag="qraw")
    qrot = cp.tile([P, NQT, 1024], BF16, tag="qrot")
    k_raw = cp.tile([P, NCT, 256], BF16, tag="kraw")
    krot = cp.tile([P, NCT, 256], BF16, tag="krot")
    oT = cp.tile([P, 8, TQ], BF16, tag="oT")
    den_sb = cp.tile([P, NQT, 2, 512], F32, tag="densb")
    y_sb = cp.tile([P, NQT, 1024], F32, tag="ysb")
    ident = cp.tile([P, P], BF16, tag="ident")
    mT0 = cp.tile([P, P], BF16, tag="mT0")
    mT4 = cp.tile([P, P], BF16, tag="mT4")
    onesb = cp.tile([P, P], F32, tag="onesb")
    eps_c = cp.tile([P, 1], F32, tag="epsc")
    ssq_q = cp.tile([P, NQT, H], F32, tag="ssqq")
    ssq_k = cp.tile([P, NCT, KV], F32, tag="ssqk")

    # ---- input DMAs ----
    nc.sync.dma_start(cosk[:], cosk_d[:])
    nc.sync.dma_start(sink[:], sink_d[:])
    nc.sync.dma_start(cosq[:], cosq_d[:])
    nc.sync.dma_start(sinq[:], sinq_d[:])
    nc.sync.dma_start(vmask[:], vmask_d[:])
    nc.sync.dma_start(xt[:], xt_d.rearrange("(a p) t -> p a t", p=P))
    nc.sync.dma_start(wk[:], wk_d.rearrange("(a p) n -> p a n", p=P))
    nc.sync.dma_start(wv[:], wv_d.rearrange("(a p) n -> p a n", p=P))
    nc.sync.dma_start(wq[:], wq_d.rearrange("(a p) n -> p a n", p=P))

    # ---- on-chip constants ----
    # identity for PE transposes
    nc.gpsimd.memset(ident[:], 0.0)
    nc.gpsimd.memset(eps_c[:], EPS)
    nc.gpsimd.affine_select(
        out=ident[:], in_=ident[:], compare_op=ALU.not_equal, fill=1.0,
        base=0, pattern=[[-1, P]], channel_multiplier=1)
    # additive sliding-window masks, applied on the PE as an extra
    # accumulate matmul (out += mT.T @ [I I I I]) inside the score group.
    # mask tile 0 keeps a < kp: add -30000 where a >= kp
    nc.gpsimd.memset(mT0[:], 0.0)
    nc.gpsimd.affine_select(
        out=mT0[:], in_=mT0[:], compare_op=ALU.is_gt, fill=-30000.0,
        base=0, pattern=[[1, P]], channel_multiplier=-1)
    # mask tile 4 keeps a >= kp: add -30000 where a < kp
    nc.gpsimd.memset(mT4[:], 0.0)
    nc.gpsimd.affine_select(
        out=mT4[:], in_=mT4[:], compare_op=ALU.is_ge, fill=-30000.0,
        base=0, pattern=[[-1, P]], channel_multiplier=1)
    # v augmentation fixed columns: the "ones" column is the per-context-tile
    # validity (0 for left-pad tiles on chunk 0) so padded keys contribute
    # nothing to the softmax denominator.
    nc.gpsimd.memset(vB[:], 0.0)
    nc.gpsimd.memset(onesb[:], 1.0)
    for g in range(KV):
        nc.vector.tensor_copy(vA[:, :, g, 64:65], vmask[:].unsqueeze(2))
        nc.vector.tensor_copy(vB[:, :, g, 32:33], vmask[:].unsqueeze(2))

    inv64 = 1.0 / 64.0

    def rmsnorm_rope(raw, rot, nt, nh, ssq, cosT, sinT, it):
        """raw/rot: [P, nt, nh*64] bf16 slabs; process tile `it`."""
        hv = raw[:, it].rearrange("p (h d) -> p h d", h=nh)
        rv = rot[:, it].rearrange("p (h d) -> p h d", h=nh)
        s2 = scr.tile([P, 1024], BF16, tag="s2")
        s2v = s2[:, 0:nh * HD].rearrange("p (h d) -> p h d", h=nh)
        nc.gpsimd.tensor_mul(s2v[:], hv[:], hv[:])
        nc.vector.tensor_reduce(
            out=ssq[:, it], in_=s2v[:], axis=mybir.AxisListType.X,
            op=ALU.add)
        # sqrt(mean + eps) in one activation, then reciprocal straight to bf16
        sq = scr.tile([P, nh], F32, tag="sq")
        nc.scalar.activation(sq[:], ssq[:, it], ACTF.Sqrt,
                             bias=eps_c[:], scale=inv64)
        rsb = scr.tile([P, nh], BF16, tag="rsb")
        with nc.allow_low_precision(reason="1/rms applied to bf16 q/k "
                                    "anyway; same final precision as f32 "
                                    "recip + bf16 copy"):
            nc.vector.reciprocal(rsb[:], sq[:])
        # normalize in place
        nc.vector.tensor_tensor(
            out=hv[:], in0=hv[:],
            in1=rsb[:].unsqueeze(2).broadcast_to([P, nh, HD]),
            op=ALU.mult)
        # rope: halves are contiguous thanks to the host evens-first permute
        yA = hv[:, :, 0:32]
        yB = hv[:, :, 32:64]
        cA = cosT[:, it:it + 1, 0:32].broadcast_to([P, nh, 32])
        cB = cosT[:, it:it + 1, 32:64].broadcast_to([P, nh, 32])
        sA = sinT[:, it:it + 1, 0:32].broadcast_to([P, nh, 32])
        sB = sinT[:, it:it + 1, 32:64].broadcast_to([P, nh, 32])
        w = nh * 32

        def half(tag):
            r = scr.tile([P, 512], BF16, tag=tag)
            return r[:, 0:w].rearrange("p (h d) -> p h d", h=nh)
        r1v, r2v, r3v, r4v = (half(t) for t in ("r1", "r2", "r3", "r4"))
        nc.vector.tensor_mul(r1v[:], yA, cA)
        nc.vector.tensor_mul(r2v[:], yB, sA)
        nc.vector.tensor_tensor(out=rv[:, :, 0:32], in0=r1v[:], in1=r2v[:],
                                op=ALU.subtract)
        nc.gpsimd.tensor_mul(r3v[:], yB, cB)
        nc.gpsimd.tensor_mul(r4v[:], yA, sB)
        nc.gpsimd.tensor_tensor(out=rv[:, :, 32:64], in0=r3v[:], in1=r4v[:],
                                op=ALU.add)

    # ---- k/v projection unit (one context tile): k and v share one PSUM
    # tile (two element-disjoint accumulation groups) so pp double-buffers
    # whole context tiles ----
    def emit_kv(ct):
        kps = pp.tile([P, 512], F32, tag="pp")
        vps = pp.tile([P, 512], F32, tag="pp")
        for dt in range(8):
            lhs = xt[:, dt, ct * P:(ct + 1) * P]
            nc.tensor.matmul(kps[:, 0:256], lhs, wk[:, dt],
                             start=(dt == 0), stop=(dt == 7))
            nc.tensor.matmul(vps[:, 0:256], lhs, wv[:, dt],
                             start=(dt == 0), stop=(dt == 7))
        nc.scalar.copy(k_raw[:, ct], kps[:, 0:256])
        # v -> vA (cols 0:64 per group) and vB (cols 64:128)
        nc.scalar.copy(
            vA[:, ct, :, 0:64],
            vps[:, 0:256].rearrange("p (g d) -> p g d", g=KV))
        nc.vector.tensor_copy(
            vB[:, ct, :, 64:128],
            vps[:, 0:256].rearrange("p (g d) -> p g d", g=KV))
        rmsnorm_rope(k_raw, krot, NCT, KV, ssq_k, cosk, sink, ct)
        # transpose krot tile -> kT
        for j2 in range(2):
            tp = stp.tile([P, P], BF16, tag="stp")
            nc.tensor.transpose(tp[:], krot[:, ct, j2 * P:(j2 + 1) * P], ident[:])
            nc.vector.tensor_copy(kT[:, j2, ct * P:(ct + 1) * P], tp[:])

    # ---- q projection helper, split into schedulable chunks ----
    def qproj_mm(at, qps0, qps1, dts):
        for dt in dts:
            lhs = xt[:, dt, TQ + at * P:TQ + (at + 1) * P]
            nc.tensor.matmul(qps0[:], lhs, wq[:, dt, 0:512],
                             start=(dt == 0), stop=(dt == 7))
            nc.tensor.matmul(qps1[:], lhs, wq[:, dt, 512:1024],
                             start=(dt == 0), stop=(dt == 7))

    def qproj_tail(at, qps0, qps1):
        nc.vector.tensor_copy(q_raw[:, at, 0:512], qps0[:])
        nc.vector.tensor_copy(q_raw[:, at, 512:1024], qps1[:])
        rmsnorm_rope(q_raw, qrot, NQT, H, ssq_q, cosq, sinq, at)

    def qproj_transpose(at, jts):
        for jt in jts:
            tp = stp.tile([P, P], BF16, tag="stp")
            nc.tensor.transpose(tp[:], qrot[:, at, jt * P:(jt + 1) * P], ident[:])
            nc.vector.tensor_copy(qT[:, jt, at * P:(at + 1) * P], tp[:])

    def emit_qproj(at):
        qps0 = pp.tile([P, 512], F32, tag="pp")
        qps1 = pp.tile([P, 512], F32, tag="pp")
        qproj_mm(at, qps0, qps1, range(8))
        qproj_tail(at, qps0, qps1)
        qproj_transpose(at, range(8))

    # ---- attention helpers ----
    # Group-batched: one matmul covers all 4 q-heads of a kv group
    # (moving operand [64, 4 heads, 128 q] = N=512). Sliding-window masks
    # are added on the PE inside the score accumulation group.
    e_const = None
    if stage in (5, 6):
        e_const = epool.tile([P, 5, 4, P], BF16, tag='e')
        nc.gpsimd.memset(e_const[:], 0.001)

    def pair_params(qb, g):
        par = g % 2
        return (par * 64,                 # partition offset of the 64 q-dims
                0 if g < 2 else 4,        # jlo: q slots 2*(jlo+i)+par
                64 if par == 0 else 32)   # denominator row in nT

    def emit_scores(qb, g):
        off, jlo, dp = pair_params(qb, g)
        e = epool.tile([P, 5, 4, P], BF16, tag="e")
        identb4 = ident[:].unsqueeze(1).broadcast_to([P, 4, P])
        for s_ in range(5):
            kt = qb + s_
            masked = s_ in (0, 4)
            sT = stp.tile([P, 512], F32, tag="stp")
            nc.tensor.matmul(
                sT[:],
                kT[off:off + 64, g // 2, kt * P:(kt + 1) * P],
                qT[off:off + 64, jlo:jlo + 4, qb * P:(qb + 1) * P],
                start=True, stop=not masked)
            if masked:
                nc.tensor.matmul(
                    sT[:].rearrange("p (h a) -> p h a", h=4),
                    mT0[:] if s_ == 0 else mT4[:], identb4,
                    start=False, stop=True)
            nc.scalar.activation(
                e[:, s_], sT[:].rearrange("p (h a) -> p h a", h=4),
                ACTF.Exp, bias=0.0, scale=float(HD) ** -0.5)
        return e

    def emit_attnv(qb, g, e):
        off, jlo, dp = pair_params(qb, g)
        par = g % 2
        nT = ntp.tile([P, 512], F32, tag="ntp")
        for s_ in range(5):
            kt = qb + s_
            if par == 0:
                lhsT = vA[:, kt, g]      # [128, 65] -> out rows 0..64
            else:
                lhsT = vB[:, kt, g]      # [128, 128] -> v at 64.., den@32
            nc.tensor.matmul(
                nT[0:lhsT.shape[-1], :], lhsT,
                e[:, s_].rearrange("p h a -> p (h a)"),
                start=(s_ == 0), stop=(s_ == 4))
        # extract denominator row + unnormalized numerator (both DVE, off
        # the Act queue); normalization happens per-qb, later
        nc.vector.tensor_copy(den_sb[dp:dp + 1, qb, g // 2, :],
                              nT[dp:dp + 1, :])
        nc.vector.tensor_copy(
            oT[off:off + 64, jlo:jlo + 4, qb * P:(qb + 1) * P],
            nT[off:off + 64, :].rearrange("p (h a) -> p h a", h=4))

    def emit_normalize(qb, g):
        # reciprocal of the denominator row, broadcast to the 64 head-dim
        # rows via a K=1 f32r outer product, scale oT in place
        off, jlo, dp = pair_params(qb, g)
        rsb = rpool.tile([P, 512], F32R, tag="rsb")
        with nc.allow_low_precision(reason="f32r keeps the f32 exponent and "
                                    "24-bit stream; fine for 1/den"):
            nc.vector.reciprocal(rsb[dp:dp + 1, :],
                                 den_sb[dp:dp + 1, qb, g // 2, :])
        rb = pp.tile([P, 512], F32, tag="pp")
        nc.tensor.matmul(
            rb[:], onesb[dp:dp + 1, 0:P].bitcast(F32R),
            rsb[dp:dp + 1, :], start=True, stop=True)
        nc.vector.tensor_mul(
            oT[off:off + 64, jlo:jlo + 4, qb * P:(qb + 1) * P],
            oT[off:off + 64, jlo:jlo + 4, qb * P:(qb + 1) * P],
            rb[off:off + 64, :].rearrange("p (h a) -> p h a", h=4))

    def emit_outproj_dh(at, dh):
        yps = pp.tile([P, 512], F32, tag="pp")
        for jt in range(8):
            nc.tensor.matmul(
                yps[:],
                oT[:, jt, at * P:(at + 1) * P],
                wo[:, jt, dh * 512:(dh + 1) * 512],
                start=(jt == 0), stop=(jt == 7))
        nc.vector.tensor_copy(y_sb[:, at, dh * 512:(dh + 1) * 512], yps[:])

    # ---- interleaved schedule ----
    # kv tiles run upfront (cheap phase, Act mostly idle); during attention
    # the scores feed the Act exp queue while PE gaps are filled with the
    # next q projection and the previous block's out-projection.
    do_q = stage >= 1
    do_attn = stage >= 2 and stage != 6
    do_attnv = stage in (2, 3, 5, 6)
    do_norm = stage in (2, 3)
    do_out = stage == 3

    def sc(qb, g):
        if do_attn:
            return emit_scores(qb, g)
        return e_const

    if stage >= 1:
        for ct in range(NCT):
            emit_kv(ct)
    if stage >= 2:
        nc.sync.dma_start(wo[:], wo_d.rearrange("(a p) n -> p a n", p=P))
    if do_q:
        emit_qproj(0)
        if stage < 2:
            for at in range(1, NQT):
                emit_qproj(at)

    for qb in range(NQT if stage >= 2 else 0):
        nxt = qb + 1
        qp = None
        if do_q and nxt < NQT:
            qpa = pp.tile([P, 512], F32, tag="pp")
            qpb = pp.tile([P, 512], F32, tag="pp")
            qp = (qpa, qpb)
        es = [None] * KV
        es[0] = sc(qb, 0)
        if qp:
            qproj_mm(nxt, *qp, dts=range(0, 4))
        es[1] = sc(qb, 1)
        if do_attnv:
            emit_attnv(qb, 0, e_const if stage in (5, 6) else es[0])
        if qp:
            qproj_mm(nxt, *qp, dts=range(4, 8))
            qproj_tail(nxt, *qp)
        es[2] = sc(qb, 2)
        if do_attnv:
            emit_attnv(qb, 1, e_const if stage in (5, 6) else es[1])
        if do_out and qb > 0:
            emit_outproj_dh(qb - 1, 0)
        es[3] = sc(qb, 3)
        if do_attnv:
            emit_attnv(qb, 2, e_const if stage in (5, 6) else es[2])
        if do_out and qb > 0:
            emit_outproj_dh(qb - 1, 1)
        if qp:
            qproj_transpose(nxt, range(8))
        if do_attnv:
            emit_attnv(qb, 3, e_const if stage in (5, 6) else es[3])
        if do_norm:
            for g in range(KV):
                emit_normalize(qb, g)
    if do_out:
        emit_outproj_dh(NQT - 1, 0)
        emit_outproj_dh(NQT - 1, 1)

    if stage == 3:
        nc.sync.dma_start(y_d.rearrange("(a p) n -> p a n", p=P), y_sb[:])
    else:
        # timing-only variants: keep the output transfer but source bytes
        # from xt (y_sb is never written below stage 3)
        nc.sync.dma_start(y_d.rearrange("(a p) n -> p a n", p=P),
                          xt[:].bitcast(F32))


def build_program(loop_n=None):
    """loop_n: if given, wrap the whole kernel body in a hardware For_i loop
    executing it loop_n times back-to-back (used by test.py to measure true
    per-execution device time via the two-point slope method, which cancels
    the fixed multi-ms dispatch/tunnel latency of this environment)."""
    nc = bass.Bass()
    with tile.TileContext(nc) as tc:
        with ExitStack() as ctx:
            if loop_n is None:
                emit(nc, tc, ctx)
            else:
                with tc.For_i(0, loop_n, 1, hint_engines=(
                        mybir.EngineType.PE, mybir.EngineType.Activation,
                        mybir.EngineType.DVE, mybir.EngineType.Pool,
                        mybir.EngineType.SP)):
                    emit(nc, tc, ctx)
    split_multiwaits(nc)
    return nc


_NC = None


def _get_program():
    global _NC
    if _NC is None:
        _NC = build_program()
    return _NC


# ---------------- host-side prep ----------------

def prep_core_inputs(x, wq, wk, wv, wo, qn_w, kn_w):
    bf = ml_dtypes.bfloat16
    perm = np.concatenate([np.arange(0, 64, 2), np.arange(1, 64, 2)])

    wq_p = np.ascontiguousarray(
        wq.reshape(D, H, HD)[:, HEAD_OF_SLOT][:, :, perm].reshape(D, H * HD)
    ).astype(bf)
    wk_p = np.ascontiguousarray(
        wk.reshape(D, KV, HD)[:, :, perm].reshape(D, KV * HD)).astype(bf)
    wv_p = np.ascontiguousarray(wv).astype(bf)
    wo_p = np.ascontiguousarray(
        wo.reshape(H, HD, D)[HEAD_OF_SLOT].reshape(H * HD, D)).astype(bf)

    inv_freq = 1.0 / (10000.0 ** (np.arange(0, HD, 2, dtype=np.float64) / HD))
    freq64 = np.concatenate([inv_freq, inv_freq])  # emb[t, d] = t * freq64[d]

    def rope_tables(tvec, w):
        ang = tvec[:, None].astype(np.float64) * freq64[None, :]
        c = np.cos(ang).astype(np.float32)
        s_ = np.sin(ang).astype(np.float32)
        we, wo_ = w[0::2], w[1::2]
        cosT = np.concatenate([we[None] * c[:, 0::2], wo_[None] * c[:, 1::2]], axis=1)
        sinT = np.concatenate([wo_[None] * s_[:, 0::2], we[None] * s_[:, 1::2]], axis=1)
        return cosT, sinT

    in_maps = []
    for core in range(NCORES):
        b, ci = divmod(core, NCORES // B)
        q_lo = ci * TQ
        c_lo = q_lo - WINDOW
        ctx_blk = np.zeros((TC, D), np.float32)
        lo = max(c_lo, 0)
        ctx_blk[lo - c_lo:, :] = x[b, lo:q_lo + TQ, :]
        xt_c = np.ascontiguousarray(ctx_blk.T).astype(bf)

        tq = np.arange(q_lo, q_lo + TQ)
        cq, sq = rope_tables(tq, qn_w)
        cosq_c = np.ascontiguousarray(
            cq.reshape(NQT, P, HD).transpose(1, 0, 2)).astype(bf)
        sinq_c = np.ascontiguousarray(
            sq.reshape(NQT, P, HD).transpose(1, 0, 2)).astype(bf)
        tk = np.arange(c_lo, c_lo + TC)
        ck, sk = rope_tables(tk, kn_w)
        cosk_c = np.ascontiguousarray(
            ck.reshape(NCT, P, HD).transpose(1, 0, 2)).astype(bf)
        sink_c = np.ascontiguousarray(
            sk.reshape(NCT, P, HD).transpose(1, 0, 2)).astype(bf)

        vm = np.ones((P, NCT), np.float32)
        n_pad_tiles = (lo - c_lo) // P
        vm[:, :n_pad_tiles] = 0.0

        in_maps.append({
            "xt": xt_c, "wq": wq_p, "wk": wk_p, "wv": wv_p, "wo": wo_p,
            "cosq": cosq_c, "sinq": sinq_c, "cosk": cosk_c, "sink": sink_c,
            "vmask": vm,
        })
    return in_maps


def kernel(x, wq, wk, wv, wo, qn_w, kn_w):
    from concourse.bass_utils import run_bass_kernel_spmd
    in_maps = prep_core_inputs(x, wq, wk, wv, wo, qn_w, kn_w)
    nc = _get_program()
    res = run_bass_kernel_spmd(nc, in_maps, list(range(NCORES)))
    out = np.empty((B, S, D), np.float32)
    for core in range(NCORES):
        b, ci = divmod(core, NCORES // B)
        out[b, ci * TQ:(ci + 1) * TQ, :] = res.results[core]["y"]
    return out

